# revision 1
# baseline (speedup 1.0000x reference)
"""CompressedKVCache kernel for Trainium2 (8 NeuronCores, head-sharded).

Computes, per (b, h) head:
  quantize k/v rows to int4 (per-row min/max affine), scatter into a
  uint8-packed cache at [start_pos : start_pos+L), then dequantize the
  cache prefix [0 : start_pos+L) back to f32.

Sharding: H=32 heads split across 8 cores (4 heads each); everything is
independent per head, no cross-core communication.

The packed cache itself is never returned, so the [start, end) region is
quantize->dequantized entirely on-chip; only the [0, start) prefix is read
from the cache inputs.
"""

import sys

sys.path.insert(0, "/opt/trn_rl_repo")

import numpy as np
from concourse import bass, mybir
from concourse import tile
from concourse.bass_utils import run_bass_kernel_spmd

F32 = mybir.dt.float32
U8 = mybir.dt.uint8
Alu = mybir.AluOpType
Act = mybir.ActivationFunctionType
AX = mybir.AxisListType
MAGIC = float(np.float32(2.0 ** 23))
INV15 = float(np.float32(1.0 / 15.0))

B, H, L, D = 2, 32, 2048, 128
MAX_SEQ = 8192
N_CORES = 8
HC = H // N_CORES  # heads per core


def _split_multiwait(nc):
    """This container's walrus accepts only ONE sync-wait per instruction;
    Tile's tail drain (and occasionally other insts) carry several. Split
    extras into single-wait EventSemaphore insts inserted just before."""
    for fn in nc.m.functions:
        for blk in fn.blocks:
            out = []
            for ins in blk.instructions:
                si = ins.sync_info
                if si is not None and si.on_wait is not None and len(si.on_wait) > 1:
                    waits = list(si.on_wait)
                    for j, w in enumerate(waits[:-1]):
                        out.append(mybir.InstEventSemaphore(
                            name=f"{ins.name}_sw{j}", ins=[], outs=[],
                            engine=ins.engine,
                            sync_info=mybir.SyncInfo(on_wait=[w], on_update=[])))
                    si.on_wait = [waits[-1]]
                    ins.sync_info = si
                out.append(ins)
            blk.instructions = out


def _build(start_pos: int):
    """Trace the per-core Bass kernel for a given start_pos.

    Per core: xk/xv (B,HC,L,D) f32, prefix packed caches (B,HC,S,64) u8 and
    prefix scale/zero rows (B,HC,S) f32 -> ok/ov (B,HC,S+L,D) f32.
    """
    S = start_pos
    E = S + L
    CQ = L // 128            # quant row-chunks per head
    CP = S // 128            # prefix row-chunks per head
    assert L % 128 == 0 and S % 128 == 0 and E <= MAX_SEQ

    nc = bass.Bass(trn_type="TRN2")

    ins_q, ins_p, ins_sc, ins_zp, outs = {}, {}, {}, {}, {}
    for t in ("k", "v"):
        ins_q[t] = nc.dram_tensor(f"x{t}", [B, HC, L, D], F32, kind="ExternalInput")
        if S:
            ins_p[t] = nc.dram_tensor(f"p{t}", [B, HC, S, D // 2], U8, kind="ExternalInput")
            ins_sc[t] = nc.dram_tensor(f"sc{t}", [B, HC, S], F32, kind="ExternalInput")
            ins_zp[t] = nc.dram_tensor(f"zp{t}", [B, HC, S], F32, kind="ExternalInput")
        outs[t] = nc.dram_tensor(f"o{t}", [B, HC, E, D], F32, kind="ExternalOutput")

    U32 = mybir.dt.uint32
    I32 = mybir.dt.int32
    CE = CQ + CP
    with tile.TileContext(nc) as tc:
        with tc.tile_pool(name="big", bufs=3) as big, \
             tc.tile_pool(name="small", bufs=3) as small:
            INF = float(np.float32(3.4e38))
            pair = 0
            for b in range(B):
                for hh in range(HC):
                    act_deq = (pair % 3 != 2)  # offload most pairs' dequant to ACT
                    pair += 1
                    # k/v share stats tiles: k cols [0,CQ), v cols [CQ,2CQ)
                    mn = small.tile([128, 2 * CQ], F32, tag="mn")
                    mx = small.tile([128, 2 * CQ], F32, tag="mx")
                    xs, os_ = {}, {}
                    for kv, t in enumerate(("k", "v")):
                        x_dram = ins_q[t][b, hh, :, :].rearrange("(c p) d -> p c d", p=128)
                        x = big.tile([128, CQ, D], F32, tag=f"x{kv}")
                        nc.sync.dma_start(out=x[:, :, :], in_=x_dram)
                        xs[t] = x
                        os_[t] = big.tile([128, CE, D], F32, tag=f"o{kv}", name=f"o{kv}")
                        # per-chunk min/max reduces (short ops stay under the DVE drain knee)
                        for c in range(CQ):
                            nc.vector.tensor_reduce(out=mx[:, kv * CQ + c:kv * CQ + c + 1],
                                                    in_=x[:, c, :], axis=AX.X, op=Alu.max)
                            nc.vector.tensor_reduce(out=mn[:, kv * CQ + c:kv * CQ + c + 1],
                                                    in_=x[:, c, :], axis=AX.X, op=Alu.min)

                    # one stats chain for both tensors
                    scale = small.tile([128, 2 * CQ], F32, tag="scale")
                    nc.vector.tensor_tensor(out=scale[:, :], in0=mx[:, :], in1=mn[:, :], op=Alu.subtract)
                    nc.vector.tensor_scalar(out=scale[:, :], in0=scale[:, :], scalar1=INV15,
                                            scalar2=1e-8, op0=Alu.mult, op1=Alu.max)
                    rcp = small.tile([128, 2 * CQ], F32, tag="rcp")
                    nc.vector.reciprocal(out=rcp[:, :], in_=scale[:, :])
                    zero = small.tile([128, 2 * CQ], F32, tag="zero")
                    nc.vector.tensor_scalar(out=zero[:, :], in0=mn[:, :], scalar1=-1.0,
                                            scalar2=None, op0=Alu.mult)
                    nc.vector.tensor_tensor(out=zero[:, :], in0=zero[:, :], in1=rcp[:, :], op=Alu.mult)
                    if act_deq:
                        nzs = small.tile([128, 2 * CQ], F32, tag="nzs")
                        nc.vector.tensor_tensor(out=nzs[:, :], in0=zero[:, :], in1=scale[:, :], op=Alu.mult)
                        nc.vector.tensor_scalar(out=nzs[:, :], in0=nzs[:, :], scalar1=-1.0,
                                                scalar2=None, op0=Alu.mult)

                    for kv, t in enumerate(("k", "v")):
                        x, o = xs[t], os_[t]
                        cc0 = kv * CQ
                        # y+round fused: ACT Identity with i32 output (RNE convert)
                        q = big.tile([128, CQ, D], I32, tag=f"q{kv}", bufs=2)
                        for c in range(CQ):
                            nc.scalar.activation(out=q[:, c, :], in_=x[:, c, :], func=Act.Identity,
                                                 bias=zero[:, cc0 + c:cc0 + c + 1],
                                                 scale=rcp[:, cc0 + c:cc0 + c + 1])
                        if act_deq:
                            for c in range(CQ):
                                nc.scalar.activation(out=o[:, CP + c, :], in_=q[:, c, :], func=Act.Identity,
                                                     bias=nzs[:, cc0 + c:cc0 + c + 1],
                                                     scale=scale[:, cc0 + c:cc0 + c + 1])
                        else:
                            for c in range(CQ):
                                nc.vector.tensor_scalar(out=o[:, CP + c, :], in0=q[:, c, :],
                                                        scalar1=zero[:, cc0 + c:cc0 + c + 1],
                                                        scalar2=scale[:, cc0 + c:cc0 + c + 1],
                                                        op0=Alu.subtract, op1=Alu.mult)

                    # ---------------- prefix region [0, S) ----------------
                    if S:
                        sc = small.tile([128, 2 * CP], F32, tag="sc")
                        zp = small.tile([128, 2 * CP], F32, tag="zp")
                        for kv, t in enumerate(("k", "v")):
                            nc.sync.dma_start(out=sc[:, kv * CP:(kv + 1) * CP],
                                              in_=ins_sc[t][b, hh, :].rearrange("(c p) -> p c", p=128))
                            nc.sync.dma_start(out=zp[:, kv * CP:(kv + 1) * CP],
                                              in_=ins_zp[t][b, hh, :].rearrange("(c p) -> p c", p=128))
                        pnzs = small.tile([128, 2 * CP], F32, tag="pnzs")
                        nc.vector.tensor_tensor(out=pnzs[:, :], in0=zp[:, :], in1=sc[:, :], op=Alu.mult)
                        nc.vector.tensor_scalar(out=pnzs[:, :], in0=pnzs[:, :], scalar1=-1.0,
                                                scalar2=None, op0=Alu.mult)

                        for kv, t in enumerate(("k", "v")):
                            o = os_[t]
                            cc0 = kv * CP
                            pk_dram = ins_p[t][b, hh, :, :].rearrange("(c p) d -> p c d", p=128)
                            pk = big.tile([128, CP, D // 2], U8, tag=f"pk{kv}")
                            nc.sync.dma_start(out=pk[:, :, :], in_=pk_dram)
                            # u32-lane nibble unpack: lohi = [lo(64) | hi(64)] per row
                            lohi = big.tile([128, CP, D], U8, tag=f"lohi{kv}")
                            h = CP // 2
                            for g in range(2):  # split ops to stay under the DVE drain knee
                                gs = slice(g * h, (g + 1) * h)
                                pk32 = pk[:, gs, :].bitcast(U32)
                                nc.vector.tensor_scalar(out=lohi[:, gs, 0:D // 2].bitcast(U32), in0=pk32,
                                                        scalar1=0x0F0F0F0F, scalar2=None, op0=Alu.bitwise_and)
                                nc.vector.tensor_scalar(out=lohi[:, gs, D // 2:D].bitcast(U32), in0=pk32,
                                                        scalar1=4, scalar2=0x0F0F0F0F,
                                                        op0=Alu.logical_shift_right, op1=Alu.bitwise_and)
                            # dequant + interleave in one op per chunk (strided out AP)
                            for c in range(CP):
                                src = lohi[:, c, :].rearrange("p (two d) -> p two d", two=2)
                                dst = o[:, c, :].rearrange("p (d two) -> p two d", two=2)
                                nc.vector.tensor_scalar(out=dst, in0=src,
                                                        scalar1=sc[:, cc0 + c:cc0 + c + 1],
                                                        scalar2=pnzs[:, cc0 + c:cc0 + c + 1],
                                                        op0=Alu.mult, op1=Alu.add)

                    for t in ("k", "v"):
                        o_dram = outs[t][b, hh, 0:E, :].rearrange("(c p) d -> p c d", p=128)
                        nc.sync.dma_start(out=o_dram, in_=os_[t][:, :, :])

    _split_multiwait(nc)
    return nc


_CACHE = {}


def _get_nc(start_pos: int):
    if start_pos not in _CACHE:
        _CACHE[start_pos] = _build(start_pos)
    return _CACHE[start_pos]


def _install_ntff_hook_shim():
    """The agent image's antenv lacks axon_hooks; recreate it so
    run_bass_kernel_spmd(trace=True) can drive NTFF profiling."""
    import types
    if "antenv.axon_hooks" in sys.modules:
        return
    mod = types.ModuleType("antenv.axon_hooks")
    state = {"hook": None}
    try:
        from trn_agent_boot.trn_boot import _ntff_profile_via_ctypes
        state["hook"] = _ntff_profile_via_ctypes("/opt/axon/libaxon_pjrt.so")
    except Exception:
        pass
    mod.get_axon_ntff_profile_hook = lambda: state["hook"]
    mod.set_axon_ntff_profile_hook = lambda h: state.__setitem__("hook", h)
    sys.modules["antenv.axon_hooks"] = mod


def _kernel_np(k, v, k_cache, v_cache, k_scale, k_zero, v_scale, v_zero, start_pos):
    """Pure-numpy fallback for shapes the bass path doesn't handle."""
    def qp(x):
        mn = x.min(-1, keepdims=True)
        mx = x.max(-1, keepdims=True)
        scale = np.maximum((mx - mn) / np.float32(15.0), np.float32(1e-8))
        zero = -mn / scale
        q = np.clip(np.round(x / scale + zero), 0, 15).astype(np.uint8)
        return (q[..., 0::2] | (q[..., 1::2] << 4)), scale[..., 0], zero[..., 0]

    def dq(p, s, z):
        lo = (p & 15).astype(np.float32)
        hi = ((p >> 4) & 15).astype(np.float32)
        q = np.stack([lo, hi], -1).reshape(p.shape[:-1] + (p.shape[-1] * 2,))
        return (q - z[..., None]) * s[..., None]

    S = int(start_pos)
    E = S + k.shape[2]
    outs = []
    for x, cache, sc, zp in ((k, k_cache, k_scale, k_zero), (v, v_cache, v_scale, v_zero)):
        pp, ps, pz = qp(x)
        cache = cache.copy(); sc = sc.copy(); zp = zp.copy()
        cache[:, :, S:E] = pp
        sc[:, :, S:E] = ps
        zp[:, :, S:E] = pz
        outs.append(dq(cache[:, :, :E], sc[:, :, :E], zp[:, :, :E]))
    return tuple(outs)


def kernel(k, v, k_cache, v_cache, k_scale, k_zero, v_scale, v_zero, start_pos,
           _trace=False):
    k = np.asarray(k, np.float32)
    v = np.asarray(v, np.float32)
    k_cache = np.asarray(k_cache, np.uint8)
    v_cache = np.asarray(v_cache, np.uint8)
    k_scale = np.asarray(k_scale, np.float32)
    k_zero = np.asarray(k_zero, np.float32)
    v_scale = np.asarray(v_scale, np.float32)
    v_zero = np.asarray(v_zero, np.float32)
    S = int(start_pos)

    if (k.shape != (B, H, L, D) or S % 128 or S + L > MAX_SEQ):
        return _kernel_np(k, v, k_cache, v_cache, k_scale, k_zero, v_scale, v_zero, S)

    nc = _get_nc(S)
    E = S + L

    in_maps = []
    for m in range(N_CORES):
        hs = slice(m * HC, (m + 1) * HC)
        im = {
            "xk": np.ascontiguousarray(k[:, hs]),
            "xv": np.ascontiguousarray(v[:, hs]),
        }
        if S:
            im["pk"] = np.ascontiguousarray(k_cache[:, hs, :S, :])
            im["pv"] = np.ascontiguousarray(v_cache[:, hs, :S, :])
            im["sck"] = np.ascontiguousarray(k_scale[:, hs, :S])
            im["zpk"] = np.ascontiguousarray(k_zero[:, hs, :S])
            im["scv"] = np.ascontiguousarray(v_scale[:, hs, :S])
            im["zpv"] = np.ascontiguousarray(v_zero[:, hs, :S])
        in_maps.append(im)

    if _trace:
        _install_ntff_hook_shim()
    res = run_bass_kernel_spmd(nc, in_maps, list(range(N_CORES)), trace=_trace)

    k_dec = np.empty((B, H, E, D), np.float32)
    v_dec = np.empty((B, H, E, D), np.float32)
    for m in range(N_CORES):
        hs = slice(m * HC, (m + 1) * HC)
        k_dec[:, hs] = res.results[m]["ok"]
        v_dec[:, hs] = res.results[m]["ov"]
    if _trace:
        return (k_dec, v_dec), res
    return k_dec, v_dec

